# revision 28
# baseline (speedup 1.0000x reference)
"""Trainium2 Bass kernel for quantized dense layer with Hadamard rotations.

Math (see reference): y = (H2 @ (sq(H2@x) @ sq(w@H1)) @ H1)/(64*64) + bias,
where sq() is per-tensor symmetric int8 stochastic quantization.

Structure exploited: Sylvester Hadamards factor as Kronecker products
(H4096 = H32 (x) H128).  Every Hadamard application is a per-128-tile PE
matmul against an H128 constant plus a cross-tile add/sub butterfly,
emitted as single strided 3D-AP ops split between the DVE and Pool
engines.  The core GEMM runs on int8-valued operands stored as bf16
(exact: |acc| < 2^25) at full PE rate; quantized operands travel between
cores as int8 (half the collective bytes) and are upcast to bf16 by
Pool-engine casting DMAs on load.  Stochastic rounding is computed as
rint(x*scale + (0.5 - noise)) via the fp32->int32 round-to-nearest cast,
with (0.5 - noise) precomputed on the host (bf16).

Sharding (8 cores): the IN axis is split 8 ways for forward transforms +
quantization.  W side is processed first so its AllToAll issues before
the chunked activation AllGathers; a dummy AllReduce at kernel start
absorbs the collective entry barrier.  The GEMM runs weight-stationary
([f,b] output), consuming AllGather chunks as they land, with the
feature-side inverse and the low batch-butterfly bits pipelined per
chunk.  The outer H8 on features is folded into the host-side unshard.
"""
import sys, os
sys.path.insert(0, '/opt/trn_rl_repo')
import numpy as np

B, IN, F = 4096, 2048, 4096
NCORES = 8
CS = IN // NCORES      # 256  per-core IN slice
FS = F // NCORES       # 512  per-core feature block
BT = B // 128          # 32   batch tiles
KT = IN // 128         # 16   contraction tiles
QMAX = 127.0
KSTOP = int(os.environ.get("KSTOP", "9"))
NCH = int(os.environ.get("NCH", "4"))       # AllGather chunks
BCH = B // NCH                              # batch cols per AG chunk
GPF = float(os.environ.get("GPF", "0.15"))    # pool share, fwd bfly
GPFB = float(os.environ.get("GPFB", "0.12"))  # pool share, inv bfly

_cache = {}


class _StopBuild(Exception):
    pass


def _sylvester(n):
    h = np.array([[1.0]], dtype=np.float32)
    while h.shape[0] < n:
        h = np.block([[h, h], [h, -h]])
    return h


def _build():
    from concourse import bass, bacc, tile, mybir
    import concourse.bass_isa as bass_isa

    DT = mybir.dt.float32
    BF = mybir.dt.bfloat16
    I32 = mybir.dt.int32
    I8 = mybir.dt.int8
    FP = mybir.dt.float16
    A = mybir.AluOpType
    ACT = mybir.ActivationFunctionType
    npbf = mybir.dt.np(BF)
    npfp = mybir.dt.np(FP)

    nc = bacc.Bacc("TRN2", target_bir_lowering=False, debug=False,
                   num_devices=NCORES)

    xk = nc.dram_tensor("xk", [B, CS], DT, kind="ExternalInput")
    nk = nc.dram_tensor("nk", [B, CS], FP, kind="ExternalInput")   # 0.5-noise_x
    wk = nc.dram_tensor("wk", [F, CS], DT, kind="ExternalInput")   # w slice^T
    mk = nc.dram_tensor("mk", [F, CS], FP, kind="ExternalInput")   # (.5-noise_w)^T
    out = nc.dram_tensor("out", [B, FS], BF, kind="ExternalOutput")

    dm_i = nc.dram_tensor("dm_i", [1, 1], DT)
    dm_o = nc.dram_tensor("dm_o", [1, 1], DT, addr_space="Shared")
    sc_i = nc.dram_tensor("sc_i", [1, 2], DT)
    sc_o = nc.dram_tensor("sc_o", [1, 2], DT, addr_space="Shared")
    xqc = [nc.dram_tensor(f"xqc{j}", [CS, BCH], FP) for j in range(NCH)]
    xqg = [nc.dram_tensor(f"xqg{j}", [IN, BCH], FP, addr_space="Shared")
           for j in range(NCH)]
    wac = nc.dram_tensor("wac", [IN, FS], FP)                      # A2A contrib
    wblk = nc.dram_tensor("wblk", [IN, FS], FP)                    # wq[:, Fk]

    h128f_d = nc.inline_tensor(_sylvester(128), name="h128f")
    h128h_d = nc.inline_tensor(_sylvester(128).astype(npfp), name="h128h")
    h128s_d = nc.inline_tensor((_sylvester(128) / 1024.0).astype(npfp),
                               name="h128s")
    idf_d = nc.inline_tensor(np.eye(128, dtype=np.float32).astype(npfp),
                             name="idf")
    rg = [list(range(NCORES))]

    NB = 32 * CS  # 8192 free columns in a fwd big tile

    def gshare(blk, gpf):
        if blk < 1024:
            return 0
        return (int(blk * gpf) // 32) * 32

    def butterfly(nc, bufs, T, blk0, e0=0, gpf=GPF):
        """FWHT across the tile-index axis of big tensors [128, T*blk0].
        One strided 3D-AP op per (engine, add/sub) per stage."""
        n = T.bit_length() - 1
        for s in range(e0, n):
            cur, nxt = bufs(s)
            blk = blk0 << s
            gp = gshare(blk, gpf)

            def v3(t, half, lo, sz):
                return t.rearrange("p (h two x) -> p h two x",
                                   two=2, x=blk)[:, :, half, lo:lo + sz]

            for (lo, sz, eng) in ((0, blk - gp, nc.vector),
                                  (blk - gp, gp, nc.gpsimd)):
                if sz <= 0:
                    continue
                eng.tensor_tensor(v3(nxt, 0, lo, sz), v3(cur, 0, lo, sz),
                                  v3(cur, 1, lo, sz), op=A.add)
                eng.tensor_tensor(v3(nxt, 1, lo, sz), v3(cur, 0, lo, sz),
                                  v3(cur, 1, lo, sz), op=A.subtract)

    with tile.TileContext(nc) as tc:
      try:
        with tc.tile_pool(name="consts", bufs=1) as cpool:
            h128f = cpool.tile([128, 128], DT)
            h128h = cpool.tile([128, 128], FP)
            h128s = cpool.tile([128, 128], FP)
            idf = cpool.tile([128, 128], FP)
            nc.sync.dma_start(h128f[:], h128f_d[:])
            nc.sync.dma_start(h128h[:], h128h_d[:])
            nc.sync.dma_start(h128s[:], h128s_d[:])
            nc.sync.dma_start(idf[:], idf_d[:])
            # long-lived scalars (alpha survives into the GEMM phase)
            al = cpool.tile([1, 1], DT)
            alb = cpool.tile([128, 1], DT)

            # ================= forward transforms + quant =================
            with tc.tile_pool(name="fwd", bufs=2) as fp_, \
                 tc.tile_pool(name="fin", bufs=4) as fin, \
                 tc.tile_pool(name="fps", bufs=1, space="PSUM") as fps, \
                 tc.tile_pool(name="qtmp", bufs=2) as qtmp, \
                 tc.tile_pool(name="qT", bufs=4) as qTp, \
                 tc.tile_pool(name="qsc", bufs=1) as qsc:

                # dummy collective to absorb the entry barrier / cc warmup
                # (reads uninitialized dram; result unused)
                nc.gpsimd.collective_compute(
                    "AllReduce", A.max, replica_groups=rg,
                    ins=[dm_i.ap().opt()], outs=[dm_o.ap().opt()])

                def fwd_side(src_tile_ap, ntiles, side, copy_eng):
                    bigA = fp_.tile([128, NB], FP, tag="bigA",
                                    name=f"bigA{side}")
                    bigB = fp_.tile([128, NB], FP, tag="bigB",
                                    name=f"bigB{side}")
                    for o in range(ntiles):
                        t = fin.tile([128, CS], DT, tag="fin", name="fint")
                        nc.sync.dma_start(t[:], src_tile_ap(o))
                        ps = fps.tile([128, CS], DT, tag="ps", name="fpst",
                                      bufs=4)
                        nc.tensor.matmul(ps[:], h128f[:], t[:], start=True,
                                         stop=True)
                        if copy_eng == "vector" or (
                                copy_eng == "both" and o % 2 == 0):
                            nc.vector.tensor_copy(
                                bigA[:, o * CS:(o + 1) * CS], ps[:])
                        else:
                            nc.scalar.activation(
                                bigA[:, o * CS:(o + 1) * CS], ps[:],
                                ACT.Copy)
                    butterfly(nc, lambda s: (bigA, bigB) if s % 2 == 0
                              else (bigB, bigA), 32, CS)
                    return bigB  # 5 stages -> result in B

                def scale_part(big, tag, idx):
                    am = qsc.tile([128, 1], DT, tag=f"am{tag}",
                                  name=f"am{tag}")
                    nc.vector.tensor_reduce(am[:], big[:],
                                            axis=mybir.AxisListType.X,
                                            op=A.max,
                                            apply_absolute_value=True)
                    red = qsc.tile([128, 1], DT, tag=f"rd{tag}",
                                   name=f"rd{tag}")
                    nc.gpsimd.partition_all_reduce(
                        red[:], am[:], channels=128,
                        reduce_op=bass_isa.ReduceOp.absmax)
                    nc.sync.dma_start(sc_i[0:1, idx:idx + 1], red[0:1, 0:1])

                def scale_finish(tag, idx):
                    sg = qsc.tile([1, 1], DT, tag=f"sg{tag}",
                                  name=f"sg{tag}")
                    nc.sync.dma_start(sg[0:1, :], sc_o[0:1, idx:idx + 1])
                    # r = QMAX/s with one newton step
                    r0 = qsc.tile([1, 1], DT, tag=f"r0{tag}", name=f"r0{tag}")
                    nc.vector.reciprocal(r0[0:1, :], sg[0:1, :])
                    mr = qsc.tile([1, 1], DT, tag=f"mr{tag}", name=f"mr{tag}")
                    nc.vector.tensor_tensor(mr[0:1, :], sg[0:1, :],
                                            r0[0:1, :], op=A.mult)
                    tw = qsc.tile([1, 1], DT, tag=f"tw{tag}", name=f"tw{tag}")
                    nc.vector.tensor_scalar(tw[0:1, :], mr[0:1, :], -1.0, 2.0,
                                            op0=A.mult, op1=A.add)
                    r1 = qsc.tile([1, 1], DT, tag=f"r1{tag}", name=f"r1{tag}")
                    nc.vector.tensor_tensor(r1[0:1, :], r0[0:1, :],
                                            tw[0:1, :], op=A.mult)
                    r127 = qsc.tile([1, 1], DT, tag=f"rq{tag}",
                                    name=f"rq{tag}")
                    nc.vector.tensor_scalar_mul(r127[0:1, :], r1[0:1, :],
                                                QMAX)
                    rb = qsc.tile([128, 1], DT, tag=f"rb{tag}",
                                  name=f"rb{tag}")
                    nc.gpsimd.partition_broadcast(rb[:, 0:1], r127[0:1, 0:1])
                    return sg, rb

                CH = 1024   # quant chunk = 4 o-tiles

                def quant_transpose(big, rb, noise_ap, side, tiles_T,
                                    chs):
                    """quantize [128, NB] -> fp16 ints, PE-transpose
                    128-blocks into tiles_T[h][128, B] (h = col-half)."""
                    nt_ch = CH // CS  # 4
                    for ch in chs:
                        nz = qtmp.tile([128, CH], FP, tag="nz", name="nzt")
                        nc.sync.dma_start(
                            nz[:].rearrange("p (o c) -> p o c", o=nt_ch),
                            noise_ap(ch))
                        qi = qtmp.tile([128, CH], I32, tag="qi", name="qit")
                        o0 = ch * CH
                        nc.vector.scalar_tensor_tensor(
                            qi[:], big[:, o0:o0 + CH], rb[:, 0:1], nz[:],
                            op0=A.mult, op1=A.add)
                        qb = qtmp.tile([128, CH], FP, tag=f"qb{side}",
                                       name="qbt")
                        nc.scalar.copy(qb[:], qi[:])
                        for ol in range(nt_ch):
                            o = ch * nt_ch + ol
                            for h in range(2):
                                ps = fps.tile([128, 128], FP, tag="tps",
                                              name="tpst", bufs=4)
                                nc.tensor.transpose(
                                    ps[:],
                                    qb[:, ol * CS + h * 128:
                                       ol * CS + (h + 1) * 128], idf[:])
                                if (o + h) % 2:
                                    nc.scalar.copy(
                                        tiles_T[h][:, o * 128:(o + 1) * 128],
                                        ps[:])
                                else:
                                    nc.vector.tensor_copy(
                                        tiles_T[h][:, o * 128:(o + 1) * 128],
                                        ps[:])

                # ---- forward transforms, then one combined scale AR --
                wrB = fwd_side(lambda o: wk[o * 128:(o + 1) * 128, :],
                               F // 128, "w", "vector")
                scale_part(wrB, "w", 0)
                xrB = fwd_side(lambda o: xk[o * 128:(o + 1) * 128, :], BT,
                               "x", "both")
                scale_part(xrB, "x", 1)
                nc.gpsimd.collective_compute(
                    "AllReduce", A.max, replica_groups=rg,
                    ins=[sc_i.ap().opt()], outs=[sc_o.ap().opt()])

                noix = (lambda c: nk[c * 512:(c + 1) * 512, :]
                        .rearrange("(o p) c -> p o c", p=128))
                noiw = (lambda c: mk[c * 512:(c + 1) * 512, :]
                        .rearrange("(o p) r -> p o r", p=128))
                chs_per_ag = BCH // 512

                def emit_ag(j):
                    for h in range(2):
                        nc.sync.dma_start(
                            xqc[j][h * 128:(h + 1) * 128, :],
                            xT[h][:, j * BCH:(j + 1) * BCH])
                    if KSTOP >= 5:
                        nc.gpsimd.collective_compute(
                            "AllGather", A.bypass, replica_groups=rg,
                            ins=[xqc[j].ap().opt()], outs=[xqg[j].ap().opt()])

                # quant x chunk 0 -> AG1 first on the cc queue
                sgx, rbx = scale_finish("x", 1)
                xT = [qTp.tile([128, B], FP, tag="qT", name=f"xT{h}")
                      for h in range(2)]
                quant_transpose(xrB, rbx, noix, "x", xT,
                                range(chs_per_ag))
                emit_ag(0)

                # quant w -> wac -> A2A (second on the queue)
                sgw, rbw = scale_finish("w", 0)
                wT = [qTp.tile([128, B], FP, tag="qT", name=f"wT{h}")
                      for h in range(2)]
                quant_transpose(wrB, rbw, noiw, "w", wT, range(NB // CH))
                for a in range(NCORES):
                    for h in range(2):
                        nc.sync.dma_start(
                            wac[a * CS + h * 128:a * CS + (h + 1) * 128, :],
                            wT[h][:, a * FS:(a + 1) * FS])
                if KSTOP >= 5:
                    nc.gpsimd.collective_compute(
                        "AllToAll", A.bypass, replica_groups=rg,
                        ins=[wac.ap().opt()], outs=[wblk.ap().opt()])

                # remaining x chunks -> AG2..AGn
                for j in range(1, NCH):
                    quant_transpose(xrB, rbx, noix, "x", xT,
                                    range(j * chs_per_ag,
                                          (j + 1) * chs_per_ag))
                    emit_ag(j)

                # alpha = sx*sw/(QMAX^2 * 2^24)
                nc.vector.tensor_tensor(al[0:1, 0:1], sgx[0:1, 0:1],
                                        sgw[0:1, 0:1], op=A.mult)
                nc.vector.tensor_scalar_mul(
                    al[0:1, 0:1], al[0:1, 0:1],
                    float(1024.0 / (QMAX * QMAX * (1 << 24))))
                nc.gpsimd.partition_broadcast(alb[:, 0:1], al[0:1, 0:1])

            if KSTOP < 6:
                raise _StopBuild()

            # ============ GEMM-T (weight stationary, [f,b] out) ===========
            nst = (BCH // 128).bit_length() - 1     # in-chunk bfly stages
            with tc.tile_pool(name="zt", bufs=1) as ztp, \
                 tc.tile_pool(name="wst", bufs=1) as wsp, \
                 tc.tile_pool(name="gem", bufs=2) as gem:
              with tc.tile_pool(name="gps", bufs=1, space="PSUM") as gps:
                zt = [ztp.tile([128, B], FP, tag="zt", name=f"zt{g}", bufs=4)
                      for g in range(4)]
                zf = [ztp.tile([128, B], FP, tag="zf", name=f"zf{g}", bufs=4)
                      for g in range(4)]
                wsb = wsp.tile([128, KT * FS], FP, tag="ws", name="wsb")
                for kt in range(KT):
                    nc.sync.dma_start(wsb[:, kt * FS:(kt + 1) * FS],
                                      wblk[kt * 128:(kt + 1) * 128, :])

                for j in range(NCH):
                    xsb = gem.tile([128, KT * BCH], FP, tag="xs",
                                   name="xsb")
                    for kt in range(KT):
                        nc.sync.dma_start(
                            xsb[:, kt * BCH:(kt + 1) * BCH],
                            xqg[j][kt * 128:(kt + 1) * 128, :])
                    for g in range(4):
                        ps = gps.tile([128, BCH], DT, tag="gp",
                                      name=f"gpt{j}_{g}", bufs=2)
                        for kt in range(KT):
                            for q in range(BCH // 512):
                                nc.tensor.matmul(
                                    ps[:, q * 512:(q + 1) * 512],
                                    wsb[:, kt * FS + g * 128:
                                        kt * FS + (g + 1) * 128],
                                    xsb[:, kt * BCH + q * 512:
                                        kt * BCH + (q + 1) * 512],
                                    start=(kt == 0), stop=(kt == KT - 1))
                        # drain with *alpha into zt[g] (scalar engine)
                        nc.scalar.activation(
                            zt[g][:, j * BCH:(j + 1) * BCH], ps[:],
                            ACT.Copy, scale=alb[:, 0:1])
                    if KSTOP < 7:
                        continue
                    # feature inverse for this chunk: H128_f then H4_g
                    zh = []
                    for g in range(4):
                        ps = gps.tile([128, BCH], DT, tag="fh",
                                      name=f"fht{j}_{g}", bufs=1)
                        for q in range(BCH // 512):
                            nc.tensor.matmul(
                                ps[:, q * 512:(q + 1) * 512], h128h[:],
                                zt[g][:, j * BCH + q * 512:
                                      j * BCH + (q + 1) * 512],
                                start=True, stop=True)
                        t = gem.tile([128, BCH], FP, tag=f"zh{g % 2}",
                                     name=f"zht{j}_{g}")
                        if g % 2:
                            nc.scalar.copy(t[:], ps[:])
                        else:
                            nc.vector.tensor_copy(t[:], ps[:])
                        zh.append(t)
                    # H4 on g (2 stages, SBUF) -> zf[g][:, chunk]
                    zm = [gem.tile([128, BCH], FP, tag=f"zm{g % 2}",
                                   name=f"zmt{j}_{g}") for g in range(4)]
                    for g0 in (0, 2):
                        nc.vector.tensor_tensor(zm[g0][:], zh[g0][:],
                                                zh[g0 + 1][:], op=A.add)
                        nc.gpsimd.tensor_tensor(zm[g0 + 1][:], zh[g0][:],
                                                zh[g0 + 1][:], op=A.subtract)
                    for g0 in (0, 1):
                        nc.vector.tensor_tensor(
                            zf[g0][:, j * BCH:(j + 1) * BCH],
                            zm[g0][:], zm[g0 + 2][:], op=A.add)
                        nc.gpsimd.tensor_tensor(
                            zf[g0 + 2][:, j * BCH:(j + 1) * BCH],
                            zm[g0][:], zm[g0 + 2][:], op=A.subtract)
                    # batch bfly stages within the chunk (o bits 0..nst-1)
                    for g in range(4):
                        butterfly(
                            nc,
                            lambda s, g=g: (
                                (zf[g][:, j * BCH:(j + 1) * BCH],
                                 zt[g][:, j * BCH:(j + 1) * BCH])
                                if s % 2 == 0 else
                                (zt[g][:, j * BCH:(j + 1) * BCH],
                                 zf[g][:, j * BCH:(j + 1) * BCH])),
                            BCH // 128, 128, gpf=GPFB)

              if KSTOP < 8:
                  raise _StopBuild()

              # ============== batch inverse tail ==============
              src, alt = (zt, zf) if nst % 2 == 1 else (zf, zt)
              fin_ = src if (5 - nst) % 2 == 0 else alt
              with tc.tile_pool(name="inv", bufs=1) as invp, \
                   tc.tile_pool(name="gp2", bufs=1, space="PSUM") as gp2:
                  ub = invp.tile([128, BT * FS], FP, tag="ub", name="ub")

                  def transposes(g):
                      # [f,b] -> [b,f] blocks into ub; scalar copies keep
                      # the DVE free for the next g's butterfly
                      for bo in range(BT):
                          ps = gp2.tile([128, 128], FP, tag="tp",
                                        name="tpt", bufs=4)
                          nc.tensor.transpose(
                              ps[:], fin_[g][:, bo * 128:(bo + 1) * 128],
                              idf[:])
                          if bo % 2:
                              nc.scalar.copy(
                                  ub[:, bo * FS + g * 128:
                                     bo * FS + (g + 1) * 128], ps[:])
                          else:
                              nc.vector.tensor_copy(
                                  ub[:, bo * FS + g * 128:
                                     bo * FS + (g + 1) * 128], ps[:])

                  for g in range(4):
                      butterfly(nc, lambda s, g=g: (src[g], alt[g]) if
                                (s - nst) % 2 == 0 else (alt[g], src[g]),
                                32, 128, e0=nst, gpf=GPFB)
                      if g < 3:
                          transposes(g)
                  # g3: transpose + batch-H128 (stationary /1024) per bo
                  for bo in range(BT):
                      ps = gp2.tile([128, 128], FP, tag="tp", name="tpt",
                                    bufs=4)
                      nc.tensor.transpose(
                          ps[:], fin_[3][:, bo * 128:(bo + 1) * 128],
                          idf[:])
                      if bo % 2:
                          nc.scalar.copy(
                              ub[:, bo * FS + 3 * 128:
                                 bo * FS + 4 * 128], ps[:])
                      else:
                          nc.vector.tensor_copy(
                              ub[:, bo * FS + 3 * 128:
                                 bo * FS + 4 * 128], ps[:])
                      ps2 = gp2.tile([128, FS], DT, tag="bp", name="bpt",
                                     bufs=2)
                      nc.tensor.matmul(ps2[:], h128s[:],
                                       ub[:, bo * FS:(bo + 1) * FS],
                                       start=True, stop=True)
                      ob = invp.tile([128, FS], BF, tag="ob", name="obt",
                                     bufs=4)
                      if bo % 2:
                          nc.vector.tensor_copy(ob[:], ps2[:])
                      else:
                          nc.scalar.copy(ob[:], ps2[:])
                      nc.sync.dma_start(
                          out[bo * 128:(bo + 1) * 128, :], ob[:])
      except _StopBuild:
        pass
    nc.compile()
    return nc


def make_in_maps(inputs):
    import ml_dtypes
    x = np.asarray(inputs["inputs"], np.float32)
    w = np.asarray(inputs["kernel"], np.float32)
    nxp = (0.5 - np.asarray(inputs["noise_x"], np.float32)).astype(
        np.float16)
    nwp = (0.5 - np.asarray(inputs["noise_w"], np.float32)).astype(
        np.float16)
    in_maps = []
    for k in range(NCORES):
        cs = slice(k * CS, (k + 1) * CS)
        in_maps.append({
            "xk": np.ascontiguousarray(x[:, cs]),
            "nk": np.ascontiguousarray(nxp[:, cs]),
            "wk": np.ascontiguousarray(w[cs, :].T),
            "mk": np.ascontiguousarray(nwp[cs, :].T),
        })
    return in_maps


def kernel(**inputs):
    from concourse.bass_utils import run_bass_kernel_spmd

    if "nc" not in _cache:
        _cache["nc"] = _build()
    nc = _cache["nc"]

    bias = np.asarray(inputs["bias"], np.float32)
    in_maps = make_in_maps(inputs)

    res = run_bass_kernel_spmd(nc, in_maps, list(range(NCORES)))
    V = np.stack([np.asarray(r["out"], np.float32) for r in res.results])
    H8 = _sylvester(8)
    # y[b, a*FS+j] = sum_a' H8[a, a'] V[a'][b, j]
    y = np.einsum("ab,bcd->cad", H8, V).reshape(B, F)
    y = y + bias[None, :]
    return y.astype(np.float32)
